# revision 8
# baseline (speedup 1.0000x reference)
"""PicoTransformerAdder Trainium2 kernel.

Math: the reference module embeds a scalar x[b,s] into d=4 dims affinely
(h = x*ew + eb), so q/k/v are affine in x.  Attention scores collapse to
score[i,j] = A*x_i*x_j + B*x_i + C*x_j + D; terms constant in j cancel in
softmax, leaving logits l_j = (a*x_i + c)*x_j.  The attention output /
pool / MLP reduce to a scalar function of

    w_i = sum_j x_j e^{t_i x_j} / sum_j e^{t_i x_j},  t = a*x + c
    W_b = mean_i w_i,  y_b = sum_h f2_h relu(W_b u_h + r_h) + f2_bias

Per batch we materialize E[j,i] = exp(a*x_j*x_i) (PE rank-1 outer product
of x with itself, exp'd on ScalarE with scale=a) and contract over j on
the PE with stationary columns {B_j, B_j*x_j} where B_j = exp(c*x_j),
since e^{t_i x_j} = e^{a x_i x_j} * B_j.

Sharding: pure data parallel, 128 batches per core across 8 cores.
"""

import numpy as np

B_FULL, S = 1024, 256
N_CORES = 8
NB = B_FULL // N_CORES  # 128 batches per core
BLK = 16                # batches per x-row DMA block
GRP = 2                 # batches per psum/exp group


def _build(a, c, u, r, f2, f2b):
    from contextlib import ExitStack

    import concourse.bacc as bacc
    import concourse.tile as tile
    from concourse import mybir
    from concourse.masks import make_identity

    f32 = mybir.dt.float32
    AF = mybir.ActivationFunctionType
    ALU = mybir.AluOpType

    nc = bacc.Bacc()
    x_in = nc.declare_dram_parameter("x", [NB, S], f32, isOutput=False)
    y_out = nc.declare_dram_parameter("y", [NB, 1], f32, isOutput=True)
    sp_dram = nc.dram_tensor("sp_scratch", [2, NB, S], f32)

    with tile.TileContext(nc) as tc, ExitStack() as ctx:
        singles = ctx.enter_context(tc.tile_pool(name="singles", bufs=1))
        xblk_pool = ctx.enter_context(tc.tile_pool(name="xblk", bufs=2))
        f_pool = ctx.enter_context(tc.tile_pool(name="f", bufs=3))
        stg_pool = ctx.enter_context(tc.tile_pool(name="stg", bufs=2))
        z_pool = ctx.enter_context(tc.tile_pool(name="z", bufs=2, space="PSUM"))
        red_pool = ctx.enter_context(tc.tile_pool(name="red", bufs=2, space="PSUM"))

        # ---------------- setup ----------------
        ident = singles.tile([128, 128], f32)
        make_identity(nc, ident)

        xrows = singles.tile([128, S], f32)
        nc.sync.dma_start(out=xrows, in_=x_in[:, :])

        # xt[:, h, b] = x[b, 128*h + s]  (x transposed, s on partitions)
        xt = singles.tile([128, 2, 128], f32)
        for h in range(2):
            pt = z_pool.tile([128, 1024], f32, tag="z")
            nc.tensor.transpose(pt[:, 0:128], xrows[:, h * 128:(h + 1) * 128], ident)
            nc.vector.tensor_copy(xt[:, h, :], pt[:, 0:128])

        # B = exp(c * x), BX = B * x (columns, s on partitions)
        bexp = singles.tile([128, 2, 128], f32)
        bx = singles.tile([128, 2, 128], f32)
        for h in range(2):
            nc.scalar.activation(bexp[:, h, :], xt[:, h, :], AF.Exp, scale=float(c))
            nc.vector.tensor_mul(bx[:, h, :], bexp[:, h, :], xt[:, h, :])

        # stationary weights: g[:, b, h, 0] = B col b of half h; [.,1] = BX col
        g = singles.tile([128, NB, 2, 2], f32)
        for h in range(2):
            nc.vector.tensor_copy(g[:, :, h, 0], bexp[:, h, :])
            nc.vector.tensor_copy(g[:, :, h, 1], bx[:, h, :])

        # constants for the output MLP
        f2t = singles.tile([128, 8], f32)
        rt = singles.tile([128, 8], f32)
        for k in range(8):
            nc.vector.memset(f2t[:, k:k + 1], float(f2[k]))
            nc.vector.memset(rt[:, k:k + 1], float(r[k]))

        # ---------------- main loop ----------------
        n_blocks = NB // BLK
        groups_per_blk = BLK // GRP
        for blk in range(n_blocks):
            xb = xblk_pool.tile([33, BLK * S], f32, tag="xb")
            src = x_in[blk * BLK:(blk + 1) * BLK, :].rearrange("b s -> (b s)")
            nc.sync.dma_start(out=xb[0:1, :], in_=src[None, :])
            nc.sync.dma_start(out=xb[32:33, :], in_=src[None, :])

            for gi in range(groups_per_blk):
                gblk = blk * groups_per_blk + gi  # global 2-batch group id
                z = z_pool.tile([128, 1024], f32, tag="z")
                for bi in range(GRP):
                    off = (gi * GRP + bi) * S
                    for h in range(2):
                        row = 32 * h
                        lhsT = xb[row:row + 1, off + 128 * h: off + 128 * h + 128]
                        rhs = xb[row:row + 1, off: off + S]
                        out = z[:, h * 512 + bi * 256: h * 512 + bi * 256 + 256]
                        nc.tensor.matmul(out, lhsT, rhs, start=True, stop=True)

                f = f_pool.tile([128, 1024], f32, tag="f")
                nc.scalar.activation(f, z, AF.Exp, scale=float(a))

                red = red_pool.tile([2, 512], f32, tag="red")
                for bi in range(GRP):
                    b = gblk * GRP + bi
                    for h in range(2):
                        rhs = f[:, h * 512 + bi * 256: h * 512 + bi * 256 + 256]
                        nc.tensor.matmul(
                            red[0:2, bi * 256:(bi + 1) * 256],
                            g[:, b, h, :], rhs,
                            start=(h == 0), stop=(h == 1),
                        )

                stg = stg_pool.tile([2, 512], f32, tag="stg")
                nc.vector.tensor_copy(stg, red)
                nc.sync.dma_start(
                    out=sp_dram[:, gblk * GRP:(gblk + 1) * GRP, :],
                    in_=stg.rearrange("p (b i) -> p b i", b=GRP),
                )

        # ---------------- epilogue ----------------
        wt = singles.tile([128, 2, S], f32)
        nc.sync.dma_start(out=wt, in_=sp_dram.rearrange("m b i -> b m i"))
        recip = singles.tile([128, S], f32)
        nc.vector.reciprocal(recip, wt[:, 0, :])
        wscr = singles.tile([128, S], f32)
        nc.vector.tensor_mul(wscr, wt[:, 1, :], recip)
        wm = singles.tile([128, 1], f32)
        nc.vector.reduce_sum(wm, wscr, axis=mybir.AxisListType.X)
        hid = singles.tile([128, 8], f32)
        for k in range(8):
            nc.scalar.activation(
                hid[:, k:k + 1], wm, AF.Relu,
                scale=float(u[k] / S), bias=rt[:, k:k + 1],
            )
        yscr = singles.tile([128, 8], f32)
        nc.vector.tensor_mul(yscr, hid, f2t)
        ysum = singles.tile([128, 1], f32)
        nc.vector.reduce_sum(ysum, yscr, axis=mybir.AxisListType.X)
        yv = singles.tile([128, 1], f32)
        nc.vector.tensor_scalar_add(yv, ysum, float(f2b))
        nc.sync.dma_start(out=y_out[:, :], in_=yv)

    nc.compile()
    return nc


def _fold_params(inputs):
    f64 = np.float64
    ew = np.asarray(inputs["emb_w"], f64)[:, 0]
    eb = np.asarray(inputs["emb_b"], f64)
    qw = np.asarray(inputs["q_w"], f64); qb = np.asarray(inputs["q_b"], f64)
    kw = np.asarray(inputs["k_w"], f64); kb = np.asarray(inputs["k_b"], f64)
    vw = np.asarray(inputs["v_w"], f64)
    vb = np.asarray(inputs["v_b"], f64)
    f1w = np.asarray(inputs["f1_w"], f64); f1b = np.asarray(inputs["f1_b"], f64)
    f2w = np.asarray(inputs["f2_w"], f64); f2b = np.asarray(inputs["f2_b"], f64)

    aq = qw @ ew; bq = qw @ eb + qb
    ak = kw @ ew; bk = kw @ eb + kb
    av = vw @ ew; bv = vw @ eb + vb
    d = ew.shape[0]
    a = float((aq @ ak) / np.sqrt(d))
    c = float((bq @ ak) / np.sqrt(d))
    u = f1w @ av
    r = f1w @ bv + f1b
    return a, c, u, r, f2w[0], float(f2b[0])


def run(inputs, trace=False, **spmd_kwargs):
    from concourse.bass_utils import run_bass_kernel_spmd

    x = np.ascontiguousarray(np.asarray(inputs["x"], np.float32))
    a, c, u, r, f2, f2b = _fold_params(inputs)
    nc = _build(a, c, u, r, f2, f2b)

    in_maps = [{"x": x[i * NB:(i + 1) * NB]} for i in range(N_CORES)]
    res = run_bass_kernel_spmd(
        nc, in_maps, core_ids=list(range(N_CORES)), trace=trace, **spmd_kwargs
    )
    y = np.concatenate([res.results[i]["y"] for i in range(N_CORES)], axis=0)
    return y.astype(np.float32), res


def kernel(**inputs):
    return run(inputs)[0]


if __name__ == "__main__":
    rng = np.random.default_rng(0)
    demo = {
        "x": rng.standard_normal((B_FULL, S), dtype=np.float32),
        "emb_w": rng.standard_normal((4, 1), dtype=np.float32) * 0.5,
        "emb_b": rng.standard_normal(4, dtype=np.float32) * 0.5,
        "q_w": rng.standard_normal((4, 4), dtype=np.float32) * 0.5,
        "q_b": rng.standard_normal(4, dtype=np.float32) * 0.5,
        "k_w": rng.standard_normal((4, 4), dtype=np.float32) * 0.5,
        "k_b": rng.standard_normal(4, dtype=np.float32) * 0.5,
        "v_w": rng.standard_normal((4, 4), dtype=np.float32) * 0.5,
        "v_b": rng.standard_normal(4, dtype=np.float32) * 0.5,
        "f1_w": rng.standard_normal((8, 4), dtype=np.float32) * 0.5,
        "f1_b": rng.standard_normal(8, dtype=np.float32) * 0.5,
        "f2_w": rng.standard_normal((1, 8), dtype=np.float32) * 0.5,
        "f2_b": rng.standard_normal(1, dtype=np.float32) * 0.5,
    }
    print(kernel(**demo)[:4])
